# revision 35
# baseline (speedup 1.0000x reference)
"""Distributed RoPE-attention kernel for 8 TRN2 NeuronCores.

Problem: x[2,2048,1024]; q/k/v/o projections (1024x1024, bias-free),
16 heads x 64 dims, RoPE on q/k, softmax attention, o-projection.

Sharding:
  - Attention: head-parallel. Core i owns heads 2i, 2i+1 (rows
    128i:128(i+1) of Wq/Wk/Wv). Each core: QKV projections (bf16) ->
    RoPE -> attention for its 2 heads over both batches, transposed
    layout [head-dim x tokens].
  - o_proj: token-parallel. Core j owns 512 tokens, 128 from each
    1024-token group: tokens 1024c + [128j, 128(j+1)) for c in 0..3.
    Six AllToAll collectives redistribute the attention outputs from
    head-sharded to token-sharded layout (the late ones one qb/128KB
    each so they never queue on the CC stream). A tiny dummy AllToAll
    -- the first gpsimd instruction -- absorbs the ~66us CC-stream
    startup latency. Each core computes
    out[:, its tokens] with the full Wo; the host reassembles.

Softmax: scores ~ N(0,1) after the 1/sqrt(Dh) scale, so exp() without
max-subtraction is safe in f32. Denominators come for free from a
ones-column appended to V (M=65 matmul costs the same as M=64).

Schedule (v2): the kernel is PE-bound overall (~345k PE cycles/core),
so the goal is a gapless PE from ~2us to the end:
  - Mini lead-in: k-proj for tokens 0:128 + q-proj for 0:512 + v for
    0:128 only -> first scores/exp at ~8us (was ~36us with half-unit
    lead-in). x DMA order feeds exactly this (cols 0:128 first).
  - All remaining QKV work is sliced into small range-closures pulled
    from a filler queue between steps; pump budget 6 early (first-qb
    production deadlines) then 1/step spreads the DVE-heavy rope work
    evenly so qb-end normalize chains (the A2A critical path) run
    within ~2us of the last PV. Deadlines verified by position
    arithmetic. Junk matmuls at t=0 warm the PE p-state ramp.
  - Normalize uses the PE ones-matmul broadcast (not the DRAM-bounce
    DMA broadcast); A2A stages + triggers pair on the gpsimd queue so
    a stage never queues behind og loads or out writes (gpsimd has no
    PSUM port, so all psum-reading copies stay on the DVE).
  - o_proj fillers for chunks 0,1 are pushed at steps 84/108, after
    the worst-case (39us-flight) landing of their A2As, so the og
    loads never head-block the PE FIFO (this stalled the baseline
    15us). Chunks 2-4 run in the tail overlapping the last A2As.

PSUM (8 banks x 2KB/partition, all in one pool, per-tag rings):
  sg scores   tag "big"  bufs=2  [128,1024]f32 -> 4 banks
  proj ranges tag "proj" bufs=1  [128,<=512]f32 -> 1 bank
  oe accum    tag "pv"   bufs=2  [128, 512]f32 -> 2 banks
  transients  tag "aux"  bufs=1  [128,<=512]f32 -> 1 bank
Ring-reuse safety: every tag's allocations are produced and consumed
in strict PE/queue program order (ranges are sequential; aux tiles are
allocated and fully consumed within a single closure; oe reuse is
safe because each qb's normalize is emitted via the queue front
before the next qb's first PV).
"""

import math
from collections import deque
import numpy as np
import ml_dtypes

import concourse.bacc as bacc
import concourse.mybir as mybir
import concourse.tile as tile
from concourse.bass_utils import run_bass_kernel_spmd

BF16 = mybir.dt.bfloat16
F32 = mybir.dt.float32
AF = mybir.ActivationFunctionType
ALU = mybir.AluOpType

N_CORES = 8
B, S, D = 2, 2048, 1024
H, DH = 16, 64
T = B * S               # 4096 tokens
HPC = H // N_CORES      # 2 heads per core
PC = HPC * DH           # 128 head-dims per core
TPC = T // N_CORES      # 512 tokens owned per core (for o_proj)

_CACHED = {}


def _rope_tables():
    inv_freq = 1.0 / (10000.0 ** (np.arange(0, DH, 2, dtype=np.float64) / DH))
    t = np.arange(S, dtype=np.float64)
    f = np.einsum("i,j->ij", t, inv_freq)          # [S, 32]
    freqs = np.concatenate([f, f], axis=-1)        # [S, 64]
    cos = np.cos(freqs).T.astype(np.float32)       # [64, S]
    sin = np.sin(freqs).T.astype(np.float32)
    cos2 = np.concatenate([cos, cos], axis=0)      # [128, S] (2 heads)
    sin2 = np.concatenate([sin, sin], axis=0)
    return cos2.astype(ml_dtypes.bfloat16), sin2.astype(ml_dtypes.bfloat16)


def _rotate_matrix_T():
    # R: per-64 block [[0,-I32],[I32,0]]  (rotate_half in column space)
    R = np.zeros((PC, PC), dtype=np.float32)
    for h in range(HPC):
        b0 = h * DH
        for i in range(32):
            R[b0 + i, b0 + 32 + i] = -1.0
            R[b0 + 32 + i, b0 + i] = 1.0
    return R.T.copy().astype(ml_dtypes.bfloat16)   # lhsT for PE


def build():
    nc = bacc.Bacc("TRN2", target_bir_lowering=False, debug=False,
                   num_devices=N_CORES)

    # weights arrive host-pre-laid in SBUF layout [128, c, m] flattened to
    # [128, c*m] so the DMA is contiguous 2KB+ lines per partition.
    xT = nc.declare_dram_parameter("xT", [D, T], BF16, isOutput=False)
    wq = nc.declare_dram_parameter("wq", [128, (D // 128) * PC], BF16, isOutput=False)
    wk = nc.declare_dram_parameter("wk", [128, (D // 128) * PC], BF16, isOutput=False)
    wv = nc.declare_dram_parameter("wv", [128, (D // 128) * PC], BF16, isOutput=False)
    wo = nc.declare_dram_parameter("wo", [128, (D // 128) * D], BF16, isOutput=False)
    out = nc.declare_dram_parameter("out", [D, TPC], F32, isOutput=True)

    cos_np, sin_np = _rope_tables()
    cos_d = nc.inline_tensor(cos_np, "cos_d")
    sin_d = nc.inline_tensor(sin_np, "sin_d")
    rt_d = nc.inline_tensor(_rotate_matrix_T(), "rt_d")
    id_d = nc.inline_tensor(np.eye(128, dtype=np.float32).astype(ml_dtypes.bfloat16), "id_d")
    ones_d = nc.inline_tensor(np.ones((1, DH), dtype=np.float32).astype(ml_dtypes.bfloat16), "ones_d")

    DC = D // 128           # 8 contraction chunks
    NQB = 4                 # 512-token query blocks per batch
    QB = S // NQB           # 512
    NKB = S // 128          # 16 key chunks per batch
    VW = HPC * (DH + 1)     # 130: packed v-normal layout (64 dims + ones) x 2

    with tile.TileContext(nc) as tc:
        with (
            tc.tile_pool(name="const", bufs=1) as constp,
            tc.tile_pool(name="resid", bufs=1) as resid,
            tc.tile_pool(name="rope", bufs=4) as ropep,
            tc.tile_pool(name="pp", bufs=6) as pp,
            tc.tile_pool(name="ogp", bufs=2) as ogp,
            tc.tile_pool(name="finp", bufs=8) as finp,
            tc.tile_pool(name="recp", bufs=4) as recp,
            tc.tile_pool(name="ps", bufs=1, space="PSUM") as psp,
            tc.tile_pool(name="dram", bufs=1, space="DRAM") as dram,
        ):
            wq_sb = constp.tile([128, DC, PC], BF16, name="wq_sb")
            wk_sb = constp.tile([128, DC, PC], BF16, name="wk_sb")
            wv_sb = constp.tile([128, DC, PC], BF16, name="wv_sb")
            wo_sb = constp.tile([128, DC, D], BF16)
            x_sb = resid.tile([128, DC, T], BF16)
            x_re = xT.ap().rearrange("(c p) m -> p c m", p=128)
            cos_sb = constp.tile([128, S], BF16)
            sin_sb = constp.tile([128, S], BF16)
            rt_sb = constp.tile([128, PC], BF16)
            id_sb = constp.tile([128, 128], BF16)
            ones_sb = constp.tile([1, DH], BF16)

            qT_sb = resid.tile([128, T], BF16)
            kT_sb = resid.tile([128, T], BF16)
            vT_sb = resid.tile([128, T], BF16)
            # v in normal layout [token-part, (64 v-dims + ones-col) x 2 heads]
            vn_sb = resid.tile([128, T // 128, VW], BF16, name="vn_sb")
            outT_sb = resid.tile([128, T], BF16)

            # ---- AllToAll buffers. Chunk c's columns CB[c]:CB[c]+w of out
            # hold tokens base_c + [w_c*j, w_c*(j+1)) on core j. The late
            # chunks are one qb each (128KB) so the tail collectives trigger
            # as soon as their qb finishes and never queue on the CC stream.
            CHUNKS = [(0, 128), (1024, 128), (2048, 64), (2560, 64),
                      (3072, 64), (3584, 64)]
            CB = [0, 128, 256, 320, 384, 448]  # out column base per chunk
            GQB_CH = {0: 0, 1: 0, 2: 1, 3: 1, 4: 2, 5: 3, 6: 4, 7: 5}
            a2a_in = [dram.tile([128 * N_CORES, w], BF16, name=f"a2a_in{c}")
                      for c, (_, w) in enumerate(CHUNKS)]
            a2a_out = [dram.tile([128 * N_CORES, w], BF16, name=f"a2a_out{c}")
                       for c, (_, w) in enumerate(CHUNKS)]
            wcc_in = dram.tile([N_CORES, 64], BF16, name="wcc_in")
            wcc_out = dram.tile([N_CORES, 64], BF16, name="wcc_out")
            # tiny dummy collective as the FIRST gpsimd instruction: the CC
            # stream takes ~66us to come up after its first trigger, so fire
            # it as early as possible.
            nc.gpsimd.collective_compute(
                "AllToAll", ALU.bypass,
                replica_groups=[list(range(N_CORES))],
                ins=[wcc_in.opt()], outs=[wcc_out.opt()],
            )
            nc.gpsimd.memset(vn_sb[:], 1.0)
            warm = recp.tile([1, 2], F32, tag="dsb", name="warm")
            nc.gpsimd.memset(warm[:], 0.0)

            # ---- DMA lead-in. The mini lead-in (k tokens 0:128, q 0:512,
            # v 0:128) needs wk + x cols 0:128 first; queues are arranged so
            # the first matmul unblocks ~2us in.
            nc.sync.dma_start(wk_sb[:], wk.ap().rearrange("p (c m) -> p c m", c=DC))
            nc.sync.dma_start(x_sb[:, :, 0:128], x_re[:, :, 0:128])
            nc.scalar.dma_start(cos_sb[:, 0:512], cos_d[:, 0:512])
            nc.scalar.dma_start(sin_sb[:, 0:512], sin_d[:, 0:512])
            nc.scalar.dma_start(rt_sb[:], rt_d[:])
            nc.gpsimd.dma_start(wv_sb[:], wv.ap().rearrange("p (c m) -> p c m", c=DC))
            nc.scalar.dma_start(wq_sb[:], wq.ap().rearrange("p (c m) -> p c m", c=DC))
            nc.sync.dma_start(x_sb[:, 0:3, 128:512], x_re[:, 0:3, 128:512])
            nc.gpsimd.dma_start(x_sb[:, 3:6, 128:512], x_re[:, 3:6, 128:512])
            nc.scalar.dma_start(x_sb[:, 6:DC, 128:512], x_re[:, 6:DC, 128:512])
            nc.gpsimd.dma_start(id_sb[:], id_d[:])
            nc.gpsimd.dma_start(ones_sb[:], ones_d[:])
            nc.gpsimd.dma_start(x_sb[:, :, 512:1024], x_re[:, :, 512:1024])
            nc.scalar.dma_start(cos_sb[:, 512:S], cos_d[:, 512:S])
            nc.scalar.dma_start(sin_sb[:, 512:S], sin_d[:, 512:S])
            nc.sync.dma_start(x_sb[:, :, 1024:2048], x_re[:, :, 1024:2048])
            nc.gpsimd.dma_start(x_sb[:, :, 2048:3072], x_re[:, :, 2048:3072])
            nc.sync.dma_start(x_sb[:, :, 3072:4096], x_re[:, :, 3072:4096])
            nc.gpsimd.dma_start(wo_sb[:], wo.ap().rearrange("p (c m) -> p c m", c=DC))

            w_sb = {"q": wq_sb, "k": wk_sb, "v": wv_sb}

            # preload the exp table-set (~2.7us) during the DMA lead-in
            warm2 = recp.tile([1, 2], BF16, tag="recb", name="warm2")
            nc.scalar.activation(warm2[:], warm[:], AF.Exp)

            # ================= building blocks =================
            proj_ps = {}

            def emit_proj_r(nm, ts, w, d0, alloc):
                if alloc:
                    proj_ps[(nm, ts)] = psp.tile(
                        [128, w], F32, tag="proj", bufs=1, name=f"ph_{nm}{ts}")
                ph = proj_ps[(nm, ts)]
                for d in (d0, d0 + 1):
                    nc.tensor.matmul(
                        ph[:], w_sb[nm][:, d, :], x_sb[:, d, ts:ts + w],
                        start=(d == 0), stop=(d == DC - 1),
                    )

            def emit_rope_r(nm, ts, w):
                ph = proj_ps.pop((nm, ts))
                dst = qT_sb if nm == "q" else kT_sb
                raw = ropep.tile([128, w], BF16, tag="raw", name=f"raw{nm}{ts}")
                # psum->bf16 cast on ScalarE (Copy is in every ACT table set,
                # so no table reload): offloads the DVE, whose backlog during
                # the production phase delays qb-end normalizes -> A2As.
                nc.scalar.activation(raw[:], ph[:], AF.Copy)
                ss = ts % S
                tmp1 = ropep.tile([128, w], BF16, tag="t1", name=f"t1_{nm}{ts}")
                nc.vector.tensor_mul(tmp1[:], raw[:], cos_sb[:, ss:ss + w])
                rot = psp.tile([128, w], F32, tag="aux", bufs=1,
                               name=f"rot{nm}{ts}")
                nc.tensor.matmul(rot[:], rt_sb[:], raw[:])
                tmp2 = ropep.tile([128, w], BF16, tag="t2", name=f"t2_{nm}{ts}")
                nc.vector.tensor_mul(tmp2[:], rot[:], sin_sb[:, ss:ss + w])
                nc.vector.tensor_add(dst[:, ts:ts + w], tmp1[:], tmp2[:])

            def emit_v_copy_r(ts, w):
                ph = proj_ps.pop(("v", ts))
                nc.scalar.activation(vT_sb[:, ts:ts + w], ph[:], AF.Copy)

            def emit_v_trans(c0, n):
                for c in range(c0, c0 + n):
                    pt = psp.tile([128, 128], BF16, tag="aux", bufs=1,
                                  name=f"pt{c}")
                    nc.tensor.matmul(
                        pt[:], vT_sb[:, c * 128:(c + 1) * 128],
                        id_sb[:], is_transpose=True,
                    )
                    nc.vector.tensor_copy(
                        vn_sb[:, c, :].rearrange("p (h e) -> p h e", h=HPC)[:, :, 0:DH],
                        pt[:].rearrange("p (h e) -> p h e", h=HPC),
                    )

            def range_closures(nm, ts, w):
                """One token-range of a projection as small filler closures.
                PSUM tiles never outlive the range's closures."""
                cls = []
                for d0 in range(0, DC, 2):
                    cls.append(lambda nm=nm, ts=ts, w=w, d0=d0:
                               emit_proj_r(nm, ts, w, d0, d0 == 0))
                if nm == "v":
                    cls.append(lambda ts=ts, w=w: emit_v_copy_r(ts, w))
                    c0, nch = ts // 128, w // 128
                    for cc in range(0, nch, 2):
                        cls.append(lambda c0=c0, cc=cc, n=min(2, nch - cc):
                                   emit_v_trans(c0 + cc, n))
                else:
                    cls.append(lambda nm=nm, ts=ts, w=w: emit_rope_r(nm, ts, w))
                return cls

            def emit_range_now(nm, ts, w):
                for c in range_closures(nm, ts, w):
                    c()

            # -------- attention step pieces --------
            def emit_scores_exp(b, qb, kb):
                bs = b * S
                qs = bs + qb * QB
                ks = bs + kb * 128
                sg = psp.tile([128, 1024], F32, tag="big", bufs=2,
                              name=f"sg{b}{qb}{kb}")
                for h in range(HPC):
                    nc.tensor.matmul(
                        sg[:, h * QB:(h + 1) * QB],
                        kT_sb[h * DH:(h + 1) * DH, ks:ks + 128],
                        qT_sb[h * DH:(h + 1) * DH, qs:qs + QB],
                    )
                p = pp.tile([128, 1024], BF16, tag="p", name=f"p{b}{qb}{kb}")
                nc.scalar.activation(p[:], sg[:], AF.Exp,
                                     scale=1.0 / math.sqrt(DH))
                return p

            oe_cur = {}

            def emit_pv(b, qb, kb, p):
                if kb == 0:
                    oe_cur[(b, qb)] = [
                        psp.tile([128, QB], F32, tag="pv", bufs=2,
                                 name=f"oe{h}_{b}_{qb}")
                        for h in range(HPC)]
                oe = oe_cur[(b, qb)]
                kc = b * NKB + kb
                for h in range(HPC):
                    nc.tensor.matmul(
                        oe[h][0:DH + 1, :],
                        vn_sb[:, kc, h * (DH + 1):(h + 1) * (DH + 1)],
                        p[:, h * QB:(h + 1) * QB],
                        start=(kb == 0), stop=(kb == NKB - 1),
                    )

            def emit_normalize(b, qb, fast=False):
                qs = b * S + qb * QB
                oe = oe_cur.pop((b, qb))
                # Free oe early (att copy) so the next qb's PV, which reuses
                # the "pv" PSUM ring, doesn't wait on the broadcast chain.
                att = None
                if not fast:
                    att = recp.tile([128, QB], BF16, tag="att", name=f"att{b}{qb}")
                rec = {}
                for h in range(HPC):
                    if not fast:
                        nc.vector.tensor_copy(att[h * DH:(h + 1) * DH, :],
                                              oe[h][0:DH, :])
                    # NOTE: reciprocal_approx_fast (custom DVE op) must read
                    # SBUF -- feeding it the PSUM row directly silently
                    # produces garbage. Hence the dsb staging copy.
                    dsb = recp.tile([1, QB], F32, tag="dsb", name=f"dsb{b}{qb}{h}")
                    nc.vector.tensor_copy(dsb[:], oe[h][DH:DH + 1, :])
                    rec[h] = recp.tile([1, QB], F32, tag="rec", name=f"rec{b}{qb}{h}")
                    nc.vector.reciprocal_approx_fast(rec[h][:], dsb[:])
                bcs = recp.tile([128, QB], BF16, tag="bcs", name=f"bcs{b}{qb}")
                for h in range(HPC):
                    recb = recp.tile([1, QB], BF16, tag="recb", name=f"recb{b}{qb}{h}")
                    nc.vector.tensor_copy(recb[:], rec[h][:])
                    # PE ones-matmul partition-broadcast: cheap on the PE and
                    # keeps the gpsimd queue empty for prompt A2A triggers.
                    bc = psp.tile([128, QB], F32, tag="aux", bufs=1,
                                  name=f"bc{b}{qb}{h}")
                    nc.tensor.matmul(bc[0:DH, :], ones_sb[:], recb[:])
                    nc.vector.tensor_copy(bcs[h * DH:(h + 1) * DH, :],
                                          bc[0:DH, :])
                for h in range(HPC):
                    src = oe[h][0:DH, :] if fast else att[h * DH:(h + 1) * DH, :]
                    nc.vector.tensor_mul(
                        outT_sb[h * DH:(h + 1) * DH, qs:qs + QB],
                        src, bcs[h * DH:(h + 1) * DH, :])

            # -------- A2A staging / o_proj --------
            def emit_a2a_stage(gqb):
                # One DMA per qb: scatter the 512 finished tokens of outT
                # into the owning cores' blocks of the chunk's A2A input.
                # On gpsimd -- the same queue as the A2A triggers -- so the
                # stage-trigger pair orders naturally and never queues behind
                # og loads or out writes (which stay on sync).
                c = GQB_CH[gqb]
                base, w = CHUNKS[c]
                nblk = QB // w
                blk0 = (gqb * QB - base) // w
                dst = a2a_in[c][blk0 * 128:(blk0 + nblk) * 128, :].rearrange(
                    "(blk p) m -> p blk m", p=128)
                src = outT_sb[:, gqb * QB:(gqb + 1) * QB].rearrange(
                    "p (blk m) -> p blk m", blk=nblk)
                # last qb: the scalar queue is guaranteed idle after the
                # final exp, so its stage issues with zero queue delay.
                eng = nc.scalar if gqb == 7 else nc.gpsimd
                eng.dma_start(dst, src)

            def emit_a2a(c):
                nc.gpsimd.collective_compute(
                    "AllToAll", ALU.bypass,
                    replica_groups=[list(range(N_CORES))],
                    ins=[a2a_in[c].opt()], outs=[a2a_out[c].opt()],
                )

            og_sb = {}

            def emit_og_load(c):
                w = CHUNKS[c][1]
                og = ogp.tile([128, DC, w], BF16, tag="og", name=f"og{c}")
                nc.sync.dma_start(
                    og[:], a2a_out[c][:].rearrange("(c p) m -> p c m", p=128))
                og_sb[c] = og

            def emit_oproj_blk(c, ob, tag="aux", bufs=1):
                w = CHUNKS[c][1]
                acc = psp.tile([128, w], F32, tag=tag, bufs=bufs,
                               name=f"acc{c}{ob}")
                for d in range(DC):
                    nc.tensor.matmul(acc[:], wo_sb[:, d, ob * 128:(ob + 1) * 128],
                                     og_sb[c][:, d, :],
                                     start=(d == 0), stop=(d == DC - 1))
                fin = finp.tile([128, w], F32, tag="fin", name=f"fin{c}{ob}")
                nc.vector.tensor_copy(fin[:], acc[:])
                nc.sync.dma_start(
                    out[ob * 128:(ob + 1) * 128, CB[c]:CB[c] + w], fin[:])

            # ================= schedule =================
            # Warm the PE clock during the DMA lead-in: the p-state ramps to
            # full rate only after ~3us of continuous execution, so a cold
            # lead-in runs at 0.65-1.2GHz. Junk matmuls on uninitialized SBUF
            # (outputs never read; start=True resets PSUM on first real use)
            # get the ramp going while x/weights are still in flight.
            # (reads outT -- not written until the first qb-end -- so the
            # WAR deps Tile inserts are long satisfied; x/w DMAs unaffected)
            for j in range(10):
                junk = psp.tile([128, 512], F32, tag="pv", bufs=2,
                                name=f"junk{j}")
                nc.tensor.matmul(junk[:], outT_sb[:, 0:128], outT_sb[:, 0:512],
                                 start=True, stop=True, skip_group_check=True)

            # Mini lead-in: exactly what scores(0,0,0) needs, interleaved so
            # the PE chews q-projs while gpsimd casts the k rope input.
            kcls = range_closures("k", 0, 128)
            qcls = range_closures("q", 0, 512)
            for c in (kcls[0], kcls[1], kcls[2], kcls[3], qcls[0], qcls[1],
                      kcls[4], qcls[2], qcls[3], qcls[4]):
                c()

            fq = deque()

            def pump(n):
                for _ in range(n):
                    if not fq:
                        return
                    fq.popleft()()

            # Filler order chosen so each k/v chunk and q block lands just
            # before its consuming step under the 6/3/1 pump budget (verified
            # by position arithmetic: a range's last closure position must be
            # < sum of budgets before its first consuming emission). v(0,128)
            # follows k(128,384) -- PV(0) is only emitted during step 1.
            for nm, ts, w in (
                ("k", 128, 384), ("v", 0, 128), ("v", 128, 384),
                ("k", 512, 512), ("v", 512, 512),
                ("q", 512, 512),
                ("k", 1024, 512), ("v", 1024, 512),
                ("k", 1536, 512), ("v", 1536, 512),
                ("q", 1024, 512), ("q", 1536, 512),
                ("k", 2048, 512), ("v", 2048, 512),
                ("q", 2048, 512),
                ("k", 2560, 512), ("v", 2560, 512),
                ("k", 3072, 512), ("v", 3072, 512),
                ("q", 2560, 512),
                ("k", 3584, 512), ("v", 3584, 512),
                ("q", 3072, 512), ("q", 3584, 512),
            ):
                fq.extend(range_closures(nm, ts, w))

            steps = [(b, qb, kb)
                     for b in range(B) for qb in range(NQB) for kb in range(NKB)]

            def qb_done_closure(pb, pqb):
                def qb_done():
                    emit_normalize(pb, pqb)
                    gqb = pb * NQB + pqb
                    emit_a2a_stage(gqb)
                    if gqb in (1, 3, 4, 5, 6):
                        emit_a2a(GQB_CH[gqb])
                return qb_done

            def push_oproj(c):
                fq.append(lambda: emit_og_load(c))
                for ob in range(DC):
                    fq.append(lambda ob=ob: emit_oproj_blk(c, ob))

            pending = None   # (b, qb, kb, p-tile) awaiting PV emission
            for idx, (b, qb, kb) in enumerate(steps):
                # 6/step while the first-qb production deadlines demand it,
                # then 1/step: spreading fillers evenly keeps the DVE queue
                # shallow, so qb-end normalize chains (the A2A critical path)
                # run within ~2us of the last PV instead of 15-40us late.
                budget = 6 if idx < 12 else 1
                p = emit_scores_exp(b, qb, kb)
                pump(budget)
                if pending is not None:
                    emit_pv(*pending)
                    pb, pqb, pkb = pending[0], pending[1], pending[2]
                    if pkb == NKB - 1:
                        # normalize + A2A staging/trigger ride the queue front
                        # so they run promptly after the qb finishes (and
                        # before the next qb's oe reuses the "pv" ring).
                        fq.appendleft(qb_done_closure(pb, pqb))
                pending = (b, qb, kb, p)

            emit_pv(*pending)
            emit_normalize(1, 3, fast=True)
            emit_a2a_stage(7)
            emit_a2a(5)
            # ALL o_proj runs in the tail: the steps stay exp-paced (their
            # PE slack is too small for o_proj fillers anyway), and the
            # 35-40us of o_proj work exactly fills the flight time of the
            # last four collectives, so the PE never idles waiting on them.
            # og(c) is always loaded well after its A2A lands (cc0 ~136us,
            # first consumer ~224us; cc5 ~257us, consumer ~263us).
            push_oproj(0)
            push_oproj(1)
            while fq:
                fq.popleft()()

            # Tail chunks (2, 3 fill the PE while the final half-size A2A is
            # in flight; 4 follows it). Each accumulates all 8 output blocks
            # into one fin tile written by a single DMA.
            def emit_oproj_chunk_merged(c):
                w = CHUNKS[c][1]
                emit_og_load(c)
                finc = finp.tile([128, DC, w], F32, tag="finm", bufs=2,
                                 name=f"finm{c}")
                for ob in range(DC):
                    acc = psp.tile([128, w], F32, tag="pv", bufs=2,
                                   name=f"accm{c}{ob}")
                    for d in range(DC):
                        nc.tensor.matmul(
                            acc[:], wo_sb[:, d, ob * 128:(ob + 1) * 128],
                            og_sb[c][:, d, :],
                            start=(d == 0), stop=(d == DC - 1))
                    nc.vector.tensor_copy(finc[:, ob, :], acc[:])
                nc.sync.dma_start(
                    out[:, CB[c]:CB[c] + w].rearrange("(c p) m -> p c m", p=128),
                    finc[:])

            for c in (2, 3, 4):
                emit_oproj_chunk_merged(c)
            # clock-keepers: the PE otherwise idles ~6us here waiting for the
            # last A2A, dropping the HAM clock to half rate right before the
            # final o_proj. Junk matmuls hold the DVFS state up.
            for j in range(16):
                junk = psp.tile([128, 512], F32, tag="pv", bufs=2,
                                name=f"tjunk{j}")
                nc.tensor.matmul(junk[:], outT_sb[:, 0:128], outT_sb[:, 0:512],
                                 start=True, stop=True, skip_group_check=True)
            emit_oproj_chunk_merged(5)

    nc.compile()
    return nc


def _get_nc():
    if "nc" not in _CACHED:
        _CACHED["nc"] = build()
    return _CACHED["nc"]


def _prep_w(Wm):
    # [D, M] (rows = contraction dim) -> SBUF layout [128, c, m] flattened
    # to [128, c*m], contiguous per partition.
    Dd, M = Wm.shape
    return np.ascontiguousarray(
        Wm.reshape(Dd // 128, 128, M).transpose(1, 0, 2).reshape(128, -1)
    ).astype(ml_dtypes.bfloat16)


def make_in_maps(x, Wq, Wk, Wv, Wo):
    xT = np.ascontiguousarray(
        np.asarray(x, dtype=np.float32).reshape(T, D).T).astype(ml_dtypes.bfloat16)
    woT = _prep_w(np.ascontiguousarray(np.asarray(Wo, dtype=np.float32).T))
    in_maps = []
    for c in range(N_CORES):
        r0, r1 = c * PC, (c + 1) * PC
        in_maps.append({
            "xT": xT,
            "wq": _prep_w(np.ascontiguousarray(np.asarray(Wq, np.float32)[r0:r1, :].T)),
            "wk": _prep_w(np.ascontiguousarray(np.asarray(Wk, np.float32)[r0:r1, :].T)),
            "wv": _prep_w(np.ascontiguousarray(np.asarray(Wv, np.float32)[r0:r1, :].T)),
            "wo": woT,
        })
    return in_maps


def assemble(outs):
    # outs[j]: [1024, 512] f32; chunk c's column block (base CB, width w)
    # holds tokens base_c + [w*j, w*(j+1)).
    chunks = [(0, 0, 128), (1024, 128, 128), (2048, 256, 64),
              (2560, 320, 64), (3072, 384, 64), (3584, 448, 64)]
    full = np.empty((T, D), dtype=np.float32)
    for j in range(N_CORES):
        o = outs[j]
        for base, cb, w in chunks:
            full[base + w * j:base + w * (j + 1), :] = o[:, cb:cb + w].T
    return np.ascontiguousarray(full.reshape(B, S, D))


def _spot_check(x, Wq, Wk, Wv, Wo, out,
                toks=(7, 1033, 2081, 2567, 3583, 4089)):  # one per A2A chunk
    """Recompute a few output rows in numpy straight from the inputs and
    compare. Catches the (rare) flaky-run/flaky-compile corruption so the
    caller can rebuild + retry instead of returning garbage."""
    xf = np.asarray(x, np.float32).reshape(T, D)
    rot = np.concatenate([np.arange(32, 64), np.arange(0, 32)])
    sgn = np.concatenate([-np.ones(32), np.ones(32)]).astype(np.float32)
    inv_freq = 1.0 / (10000.0 ** (np.arange(0, DH, 2, dtype=np.float64) / DH))
    fr = np.concatenate([np.arange(S)[:, None] * inv_freq[None, :]] * 2, axis=1)
    cos, sin = np.cos(fr).astype(np.float32), np.sin(fr).astype(np.float32)
    for b in range(B):
        bt = [t for t in toks if t // S == b]
        if not bt:
            continue
        xb = xf[b * S:(b + 1) * S]
        k = (xb @ Wk.T).reshape(S, H, DH)
        v = (xb @ Wv.T).reshape(S, H, DH)
        k = k * cos[:, None, :] + k[:, :, rot] * (sin * sgn)[:, None, :]
        for t in bt:
            s = t - b * S
            q = (xf[t] @ Wq.T).reshape(H, DH)
            q = q * cos[s][None, :] + q[:, rot] * (sin[s] * sgn)[None, :]
            sc = np.einsum("hd,shd->hs", q, k) / np.sqrt(np.float32(DH))
            p = np.exp(sc - sc.max(axis=1, keepdims=True))
            p /= p.sum(axis=1, keepdims=True)
            att = np.einsum("hs,shd->hd", p, v).reshape(D)
            exp_row = att @ Wo.T
            got = out.reshape(T, D)[t]
            if np.linalg.norm(got - exp_row) > 0.05 * np.linalg.norm(exp_row):
                return False
    return True


def kernel(x, Wq, Wk, Wv, Wo):
    in_maps = make_in_maps(x, Wq, Wk, Wv, Wo)
    for attempt in range(2):
        nc = _get_nc()
        res = run_bass_kernel_spmd(nc, in_maps, core_ids=list(range(N_CORES)))
        outs = [res.results[c]["out"] for c in range(N_CORES)]   # [1024, 512]
        full = assemble(outs).astype(np.float32)
        if _spot_check(x, Wq, Wk, Wv, Wo, full):
            return full
        _CACHED.clear()   # flaky run or compile: rebuild and retry once
    return full


if __name__ == "__main__":
    rng = np.random.default_rng(0)
    ins = {
        "x": rng.standard_normal((B, S, D), dtype=np.float32),
        "Wq": rng.standard_normal((D, D), dtype=np.float32) / 32,
        "Wk": rng.standard_normal((D, D), dtype=np.float32) / 32,
        "Wv": rng.standard_normal((D, D), dtype=np.float32) / 32,
        "Wo": rng.standard_normal((D, D), dtype=np.float32) / 32,
    }
    o = kernel(**ins)
    print("kernel out:", o.shape, o.dtype, float(np.abs(o).mean()))


# revision 39
# speedup vs baseline: 1.0055x; 1.0055x over previous
"""Distributed RoPE-attention kernel for 8 TRN2 NeuronCores.

Problem: x[2,2048,1024]; q/k/v/o projections (1024x1024, bias-free),
16 heads x 64 dims, RoPE on q/k, softmax attention, o-projection.

Sharding:
  - Attention: head-parallel. Core i owns heads 2i, 2i+1 (rows
    128i:128(i+1) of Wq/Wk/Wv). Each core: QKV projections (bf16) ->
    RoPE -> attention for its 2 heads over both batches, transposed
    layout [head-dim x tokens].
  - o_proj: token-parallel. Core j owns 512 tokens, 128 from each
    1024-token group: tokens 1024c + [128j, 128(j+1)) for c in 0..3.
    Six AllToAll collectives redistribute the attention outputs from
    head-sharded to token-sharded layout (the late ones one qb/128KB
    each so they never queue on the CC stream). A tiny dummy AllToAll
    -- the first gpsimd instruction -- absorbs the ~66us CC-stream
    startup latency. Each core computes
    out[:, its tokens] with the full Wo; the host reassembles.

Softmax: scores ~ N(0,1) after the 1/sqrt(Dh) scale, so exp() without
max-subtraction is safe in f32. Denominators come for free from a
ones-column appended to V (M=65 matmul costs the same as M=64).

Schedule (v2): the kernel is PE-bound overall (~345k PE cycles/core),
so the goal is a gapless PE from ~2us to the end:
  - Mini lead-in: k-proj for tokens 0:128 + q-proj for 0:512 + v for
    0:128 only -> first scores/exp at ~8us (was ~36us with half-unit
    lead-in). x DMA order feeds exactly this (cols 0:128 first).
  - All remaining QKV work is sliced into small range-closures pulled
    from a filler queue between steps; pump budget 6 early (first-qb
    production deadlines) then 1/step spreads the DVE-heavy rope work
    evenly so qb-end normalize chains (the A2A critical path) run
    within ~2us of the last PV. Deadlines verified by position
    arithmetic. Junk matmuls at t=0 warm the PE p-state ramp.
  - Normalize uses the PE ones-matmul broadcast (not the DRAM-bounce
    DMA broadcast); A2A stages + triggers pair on the gpsimd queue so
    a stage never queues behind og loads or out writes (gpsimd has no
    PSUM port, so all psum-reading copies stay on the DVE).
  - o_proj fillers for chunks 0,1 are pushed at steps 84/108, after
    the worst-case (39us-flight) landing of their A2As, so the og
    loads never head-block the PE FIFO (this stalled the baseline
    15us). Chunks 2-4 run in the tail overlapping the last A2As.

PSUM (8 banks x 2KB/partition, all in one pool, per-tag rings):
  sg scores   tag "big"  bufs=2  [128,1024]f32 -> 4 banks
  proj ranges tag "proj" bufs=1  [128,<=512]f32 -> 1 bank
  oe accum    tag "pv"   bufs=2  [128, 512]f32 -> 2 banks
  transients  tag "aux"  bufs=1  [128,<=512]f32 -> 1 bank
Ring-reuse safety: every tag's allocations are produced and consumed
in strict PE/queue program order (ranges are sequential; aux tiles are
allocated and fully consumed within a single closure; oe reuse is
safe because each qb's normalize is emitted via the queue front
before the next qb's first PV).
"""

import math
from collections import deque
import numpy as np
import ml_dtypes

import concourse.bacc as bacc
import concourse.mybir as mybir
import concourse.tile as tile
from concourse.bass_utils import run_bass_kernel_spmd

BF16 = mybir.dt.bfloat16
F32 = mybir.dt.float32
AF = mybir.ActivationFunctionType
ALU = mybir.AluOpType

N_CORES = 8
B, S, D = 2, 2048, 1024
H, DH = 16, 64
T = B * S               # 4096 tokens
HPC = H // N_CORES      # 2 heads per core
PC = HPC * DH           # 128 head-dims per core
TPC = T // N_CORES      # 512 tokens owned per core (for o_proj)

_CACHED = {}


def _rope_tables():
    inv_freq = 1.0 / (10000.0 ** (np.arange(0, DH, 2, dtype=np.float64) / DH))
    t = np.arange(S, dtype=np.float64)
    f = np.einsum("i,j->ij", t, inv_freq)          # [S, 32]
    freqs = np.concatenate([f, f], axis=-1)        # [S, 64]
    cos = np.cos(freqs).T.astype(np.float32)       # [64, S]
    sin = np.sin(freqs).T.astype(np.float32)
    cos2 = np.concatenate([cos, cos], axis=0)      # [128, S] (2 heads)
    sin2 = np.concatenate([sin, sin], axis=0)
    return cos2.astype(ml_dtypes.bfloat16), sin2.astype(ml_dtypes.bfloat16)


def _rotate_matrix_T():
    # R: per-64 block [[0,-I32],[I32,0]]  (rotate_half in column space)
    R = np.zeros((PC, PC), dtype=np.float32)
    for h in range(HPC):
        b0 = h * DH
        for i in range(32):
            R[b0 + i, b0 + 32 + i] = -1.0
            R[b0 + 32 + i, b0 + i] = 1.0
    return R.T.copy().astype(ml_dtypes.bfloat16)   # lhsT for PE


def build():
    nc = bacc.Bacc("TRN2", target_bir_lowering=False, debug=False,
                   num_devices=N_CORES)

    # weights arrive host-pre-laid in SBUF layout [128, c, m] flattened to
    # [128, c*m] so the DMA is contiguous 2KB+ lines per partition.
    xT = nc.declare_dram_parameter("xT", [D, T], BF16, isOutput=False)
    wq = nc.declare_dram_parameter("wq", [128, (D // 128) * PC], BF16, isOutput=False)
    wk = nc.declare_dram_parameter("wk", [128, (D // 128) * PC], BF16, isOutput=False)
    wv = nc.declare_dram_parameter("wv", [128, (D // 128) * PC], BF16, isOutput=False)
    wo = nc.declare_dram_parameter("wo", [128, (D // 128) * D], BF16, isOutput=False)
    out = nc.declare_dram_parameter("out", [D, TPC], F32, isOutput=True)

    cos_np, sin_np = _rope_tables()
    cos_d = nc.inline_tensor(cos_np, "cos_d")
    sin_d = nc.inline_tensor(sin_np, "sin_d")
    rt_d = nc.inline_tensor(_rotate_matrix_T(), "rt_d")
    id_d = nc.inline_tensor(np.eye(128, dtype=np.float32).astype(ml_dtypes.bfloat16), "id_d")
    ones_d = nc.inline_tensor(np.ones((1, DH), dtype=np.float32).astype(ml_dtypes.bfloat16), "ones_d")

    DC = D // 128           # 8 contraction chunks
    NQB = 4                 # 512-token query blocks per batch
    QB = S // NQB           # 512
    NKB = S // 128          # 16 key chunks per batch
    VW = HPC * (DH + 1)     # 130: packed v-normal layout (64 dims + ones) x 2

    with tile.TileContext(nc) as tc:
        with (
            tc.tile_pool(name="const", bufs=1) as constp,
            tc.tile_pool(name="resid", bufs=1) as resid,
            tc.tile_pool(name="rope", bufs=4) as ropep,
            tc.tile_pool(name="pp", bufs=6) as pp,
            tc.tile_pool(name="ogp", bufs=2) as ogp,
            tc.tile_pool(name="finp", bufs=8) as finp,
            tc.tile_pool(name="recp", bufs=4) as recp,
            tc.tile_pool(name="ps", bufs=1, space="PSUM") as psp,
            tc.tile_pool(name="dram", bufs=1, space="DRAM") as dram,
        ):
            wq_sb = constp.tile([128, DC, PC], BF16, name="wq_sb")
            wk_sb = constp.tile([128, DC, PC], BF16, name="wk_sb")
            wv_sb = constp.tile([128, DC, PC], BF16, name="wv_sb")
            wo_sb = constp.tile([128, DC, D], BF16)
            x_sb = resid.tile([128, DC, T], BF16)
            x_re = xT.ap().rearrange("(c p) m -> p c m", p=128)
            cos_sb = constp.tile([128, S], BF16)
            sin_sb = constp.tile([128, S], BF16)
            rt_sb = constp.tile([128, PC], BF16)
            id_sb = constp.tile([128, 128], BF16)
            ones_sb = constp.tile([1, DH], BF16)

            qT_sb = resid.tile([128, T], BF16)
            kT_sb = resid.tile([128, T], BF16)
            vT_sb = resid.tile([128, T], BF16)
            # v in normal layout [token-part, (64 v-dims + ones-col) x 2 heads]
            vn_sb = resid.tile([128, T // 128, VW], BF16, name="vn_sb")
            outT_sb = resid.tile([128, T], BF16)

            # ---- AllToAll buffers. Chunk c's columns CB[c]:CB[c]+w of out
            # hold tokens base_c + [w_c*j, w_c*(j+1)) on core j. The late
            # chunks are one qb each (128KB) so the tail collectives trigger
            # as soon as their qb finishes and never queue on the CC stream.
            CHUNKS = [(0, 128), (1024, 128), (2048, 64), (2560, 64),
                      (3072, 64), (3584, 64)]
            CB = [0, 128, 256, 320, 384, 448]  # out column base per chunk
            GQB_CH = {0: 0, 1: 0, 2: 1, 3: 1, 4: 2, 5: 3, 6: 4, 7: 5}
            a2a_in = [dram.tile([128 * N_CORES, w], BF16, name=f"a2a_in{c}")
                      for c, (_, w) in enumerate(CHUNKS)]
            a2a_out = [dram.tile([128 * N_CORES, w], BF16, name=f"a2a_out{c}")
                       for c, (_, w) in enumerate(CHUNKS)]
            wcc_in = dram.tile([N_CORES, 64], BF16, name="wcc_in")
            wcc_out = dram.tile([N_CORES, 64], BF16, name="wcc_out")
            # tiny dummy collective as the FIRST gpsimd instruction: the CC
            # stream takes ~66us to come up after its first trigger, so fire
            # it as early as possible.
            nc.gpsimd.collective_compute(
                "AllToAll", ALU.bypass,
                replica_groups=[list(range(N_CORES))],
                ins=[wcc_in.opt()], outs=[wcc_out.opt()],
            )
            nc.gpsimd.memset(vn_sb[:], 1.0)
            warm = recp.tile([1, 2], F32, tag="dsb", name="warm")
            nc.gpsimd.memset(warm[:], 0.0)

            # ---- DMA lead-in. The mini lead-in (k tokens 0:128, q 0:512,
            # v 0:128) needs wk + x cols 0:128 first; queues are arranged so
            # the first matmul unblocks ~2us in.
            nc.sync.dma_start(wk_sb[:], wk.ap().rearrange("p (c m) -> p c m", c=DC))
            nc.sync.dma_start(x_sb[:, :, 0:128], x_re[:, :, 0:128])
            nc.scalar.dma_start(cos_sb[:, 0:512], cos_d[:, 0:512])
            nc.scalar.dma_start(sin_sb[:, 0:512], sin_d[:, 0:512])
            nc.scalar.dma_start(rt_sb[:], rt_d[:])
            nc.gpsimd.dma_start(wv_sb[:], wv.ap().rearrange("p (c m) -> p c m", c=DC))
            nc.scalar.dma_start(wq_sb[:], wq.ap().rearrange("p (c m) -> p c m", c=DC))
            nc.sync.dma_start(x_sb[:, 0:3, 128:512], x_re[:, 0:3, 128:512])
            nc.gpsimd.dma_start(x_sb[:, 3:6, 128:512], x_re[:, 3:6, 128:512])
            nc.scalar.dma_start(x_sb[:, 6:DC, 128:512], x_re[:, 6:DC, 128:512])
            nc.gpsimd.dma_start(id_sb[:], id_d[:])
            nc.gpsimd.dma_start(ones_sb[:], ones_d[:])
            nc.gpsimd.dma_start(x_sb[:, :, 512:1024], x_re[:, :, 512:1024])
            nc.scalar.dma_start(cos_sb[:, 512:S], cos_d[:, 512:S])
            nc.scalar.dma_start(sin_sb[:, 512:S], sin_d[:, 512:S])
            nc.sync.dma_start(x_sb[:, :, 1024:2048], x_re[:, :, 1024:2048])
            nc.gpsimd.dma_start(x_sb[:, :, 2048:3072], x_re[:, :, 2048:3072])
            nc.sync.dma_start(x_sb[:, :, 3072:4096], x_re[:, :, 3072:4096])
            nc.gpsimd.dma_start(wo_sb[:], wo.ap().rearrange("p (c m) -> p c m", c=DC))

            w_sb = {"q": wq_sb, "k": wk_sb, "v": wv_sb}

            # preload the exp table-set (~2.7us) during the DMA lead-in
            warm2 = recp.tile([1, 2], BF16, tag="recb", name="warm2")
            nc.scalar.activation(warm2[:], warm[:], AF.Exp)

            # ================= building blocks =================
            proj_ps = {}

            def emit_proj_r(nm, ts, w, d0, alloc):
                if alloc:
                    proj_ps[(nm, ts)] = psp.tile(
                        [128, w], F32, tag="proj", bufs=1, name=f"ph_{nm}{ts}")
                ph = proj_ps[(nm, ts)]
                for d in (d0, d0 + 1):
                    nc.tensor.matmul(
                        ph[:], w_sb[nm][:, d, :], x_sb[:, d, ts:ts + w],
                        start=(d == 0), stop=(d == DC - 1),
                    )

            def emit_rope_r(nm, ts, w):
                ph = proj_ps.pop((nm, ts))
                dst = qT_sb if nm == "q" else kT_sb
                raw = ropep.tile([128, w], BF16, tag="raw", name=f"raw{nm}{ts}")
                # psum->bf16 cast on ScalarE (Copy is in every ACT table set,
                # so no table reload): offloads the DVE, whose backlog during
                # the production phase delays qb-end normalizes -> A2As.
                nc.scalar.activation(raw[:], ph[:], AF.Copy)
                ss = ts % S
                tmp1 = ropep.tile([128, w], BF16, tag="t1", name=f"t1_{nm}{ts}")
                nc.vector.tensor_mul(tmp1[:], raw[:], cos_sb[:, ss:ss + w])
                rot = psp.tile([128, w], F32, tag="aux", bufs=1,
                               name=f"rot{nm}{ts}")
                nc.tensor.matmul(rot[:], rt_sb[:], raw[:])
                tmp2 = ropep.tile([128, w], BF16, tag="t2", name=f"t2_{nm}{ts}")
                nc.vector.tensor_mul(tmp2[:], rot[:], sin_sb[:, ss:ss + w])
                nc.vector.tensor_add(dst[:, ts:ts + w], tmp1[:], tmp2[:])

            def emit_v_copy_r(ts, w):
                ph = proj_ps.pop(("v", ts))
                nc.scalar.activation(vT_sb[:, ts:ts + w], ph[:], AF.Copy)

            def emit_v_trans(c0, n):
                for c in range(c0, c0 + n):
                    pt = psp.tile([128, 128], BF16, tag="aux", bufs=1,
                                  name=f"pt{c}")
                    nc.tensor.matmul(
                        pt[:], vT_sb[:, c * 128:(c + 1) * 128],
                        id_sb[:], is_transpose=True,
                    )
                    nc.vector.tensor_copy(
                        vn_sb[:, c, :].rearrange("p (h e) -> p h e", h=HPC)[:, :, 0:DH],
                        pt[:].rearrange("p (h e) -> p h e", h=HPC),
                    )

            def range_closures(nm, ts, w):
                """One token-range of a projection as small filler closures.
                PSUM tiles never outlive the range's closures."""
                cls = []
                for d0 in range(0, DC, 2):
                    cls.append(lambda nm=nm, ts=ts, w=w, d0=d0:
                               emit_proj_r(nm, ts, w, d0, d0 == 0))
                if nm == "v":
                    cls.append(lambda ts=ts, w=w: emit_v_copy_r(ts, w))
                    c0, nch = ts // 128, w // 128
                    for cc in range(0, nch, 2):
                        cls.append(lambda c0=c0, cc=cc, n=min(2, nch - cc):
                                   emit_v_trans(c0 + cc, n))
                else:
                    cls.append(lambda nm=nm, ts=ts, w=w: emit_rope_r(nm, ts, w))
                return cls

            def emit_range_now(nm, ts, w):
                for c in range_closures(nm, ts, w):
                    c()

            # -------- attention step pieces --------
            def emit_scores_exp(b, qb, kb):
                bs = b * S
                qs = bs + qb * QB
                ks = bs + kb * 128
                sg = psp.tile([128, 1024], F32, tag="big", bufs=2,
                              name=f"sg{b}{qb}{kb}")
                for h in range(HPC):
                    nc.tensor.matmul(
                        sg[:, h * QB:(h + 1) * QB],
                        kT_sb[h * DH:(h + 1) * DH, ks:ks + 128],
                        qT_sb[h * DH:(h + 1) * DH, qs:qs + QB],
                    )
                p = pp.tile([128, 1024], BF16, tag="p", name=f"p{b}{qb}{kb}")
                nc.scalar.activation(p[:], sg[:], AF.Exp,
                                     scale=1.0 / math.sqrt(DH))
                return p

            oe_cur = {}

            def emit_pv(b, qb, kb, p):
                if kb == 0:
                    oe_cur[(b, qb)] = [
                        psp.tile([128, QB], F32, tag="pv", bufs=2,
                                 name=f"oe{h}_{b}_{qb}")
                        for h in range(HPC)]
                oe = oe_cur[(b, qb)]
                kc = b * NKB + kb
                for h in range(HPC):
                    nc.tensor.matmul(
                        oe[h][0:DH + 1, :],
                        vn_sb[:, kc, h * (DH + 1):(h + 1) * (DH + 1)],
                        p[:, h * QB:(h + 1) * QB],
                        start=(kb == 0), stop=(kb == NKB - 1),
                    )

            def emit_normalize(b, qb, fast=False):
                qs = b * S + qb * QB
                oe = oe_cur.pop((b, qb))
                # Free oe early (att copy) so the next qb's PV, which reuses
                # the "pv" PSUM ring, doesn't wait on the broadcast chain.
                att = None
                if not fast:
                    att = recp.tile([128, QB], BF16, tag="att", name=f"att{b}{qb}")
                rec = {}
                for h in range(HPC):
                    if not fast:
                        nc.vector.tensor_copy(att[h * DH:(h + 1) * DH, :],
                                              oe[h][0:DH, :])
                    # NOTE: reciprocal_approx_fast (custom DVE op) must read
                    # SBUF -- feeding it the PSUM row directly silently
                    # produces garbage. Hence the dsb staging copy.
                    dsb = recp.tile([1, QB], F32, tag="dsb", name=f"dsb{b}{qb}{h}")
                    nc.vector.tensor_copy(dsb[:], oe[h][DH:DH + 1, :])
                    rec[h] = recp.tile([1, QB], F32, tag="rec", name=f"rec{b}{qb}{h}")
                    nc.vector.reciprocal_approx_fast(rec[h][:], dsb[:])
                bcs = recp.tile([128, QB], BF16, tag="bcs", name=f"bcs{b}{qb}")
                for h in range(HPC):
                    recb = recp.tile([1, QB], BF16, tag="recb", name=f"recb{b}{qb}{h}")
                    nc.vector.tensor_copy(recb[:], rec[h][:])
                    # PE ones-matmul partition-broadcast: cheap on the PE and
                    # keeps the gpsimd queue empty for prompt A2A triggers.
                    bc = psp.tile([128, QB], F32, tag="aux", bufs=1,
                                  name=f"bc{b}{qb}{h}")
                    nc.tensor.matmul(bc[0:DH, :], ones_sb[:], recb[:])
                    nc.vector.tensor_copy(bcs[h * DH:(h + 1) * DH, :],
                                          bc[0:DH, :])
                for h in range(HPC):
                    src = oe[h][0:DH, :] if fast else att[h * DH:(h + 1) * DH, :]
                    nc.vector.tensor_mul(
                        outT_sb[h * DH:(h + 1) * DH, qs:qs + QB],
                        src, bcs[h * DH:(h + 1) * DH, :])

            # -------- A2A staging / o_proj --------
            def emit_a2a_stage(gqb):
                # One DMA per qb: scatter the 512 finished tokens of outT
                # into the owning cores' blocks of the chunk's A2A input.
                # On gpsimd -- the same queue as the A2A triggers -- so the
                # stage-trigger pair orders naturally and never queues behind
                # og loads or out writes (which stay on sync).
                c = GQB_CH[gqb]
                base, w = CHUNKS[c]
                nblk = QB // w
                blk0 = (gqb * QB - base) // w
                dst = a2a_in[c][blk0 * 128:(blk0 + nblk) * 128, :].rearrange(
                    "(blk p) m -> p blk m", p=128)
                src = outT_sb[:, gqb * QB:(gqb + 1) * QB].rearrange(
                    "p (blk m) -> p blk m", blk=nblk)
                nc.gpsimd.dma_start(dst, src)

            def emit_a2a(c):
                nc.gpsimd.collective_compute(
                    "AllToAll", ALU.bypass,
                    replica_groups=[list(range(N_CORES))],
                    ins=[a2a_in[c].opt()], outs=[a2a_out[c].opt()],
                )

            og_sb = {}

            def emit_og_load(c):
                w = CHUNKS[c][1]
                og = ogp.tile([128, DC, w], BF16, tag="og", name=f"og{c}")
                nc.sync.dma_start(
                    og[:], a2a_out[c][:].rearrange("(c p) m -> p c m", p=128))
                og_sb[c] = og

            def emit_oproj_blk(c, ob, tag="aux", bufs=1):
                w = CHUNKS[c][1]
                acc = psp.tile([128, w], F32, tag=tag, bufs=bufs,
                               name=f"acc{c}{ob}")
                for d in range(DC):
                    nc.tensor.matmul(acc[:], wo_sb[:, d, ob * 128:(ob + 1) * 128],
                                     og_sb[c][:, d, :],
                                     start=(d == 0), stop=(d == DC - 1))
                fin = finp.tile([128, w], F32, tag="fin", name=f"fin{c}{ob}")
                nc.vector.tensor_copy(fin[:], acc[:])
                nc.sync.dma_start(
                    out[ob * 128:(ob + 1) * 128, CB[c]:CB[c] + w], fin[:])

            # ================= schedule =================
            # Warm the PE clock during the DMA lead-in: the p-state ramps to
            # full rate only after ~3us of continuous execution, so a cold
            # lead-in runs at 0.65-1.2GHz. Junk matmuls on uninitialized SBUF
            # (outputs never read; start=True resets PSUM on first real use)
            # get the ramp going while x/weights are still in flight.
            # (reads outT -- not written until the first qb-end -- so the
            # WAR deps Tile inserts are long satisfied; x/w DMAs unaffected)
            for j in range(10):
                junk = psp.tile([128, 512], F32, tag="pv", bufs=2,
                                name=f"junk{j}")
                nc.tensor.matmul(junk[:], outT_sb[:, 0:128], outT_sb[:, 0:512],
                                 start=True, stop=True, skip_group_check=True)

            # Mini lead-in: exactly what scores(0,0,0) needs, interleaved so
            # the PE chews q-projs while gpsimd casts the k rope input.
            kcls = range_closures("k", 0, 128)
            qcls = range_closures("q", 0, 512)
            for c in (kcls[0], kcls[1], kcls[2], kcls[3], qcls[0], qcls[1],
                      kcls[4], qcls[2], qcls[3], qcls[4]):
                c()

            fq = deque()

            def pump(n):
                for _ in range(n):
                    if not fq:
                        return
                    fq.popleft()()

            # Filler order chosen so each k/v chunk and q block lands just
            # before its consuming step under the 6/3/1 pump budget (verified
            # by position arithmetic: a range's last closure position must be
            # < sum of budgets before its first consuming emission). v(0,128)
            # follows k(128,384) -- PV(0) is only emitted during step 1.
            for nm, ts, w in (
                ("k", 128, 384), ("v", 0, 128), ("v", 128, 384),
                ("k", 512, 512), ("v", 512, 512),
                ("q", 512, 512),
                ("k", 1024, 512), ("v", 1024, 512),
                ("k", 1536, 512), ("v", 1536, 512),
                ("q", 1024, 512), ("q", 1536, 512),
                ("k", 2048, 512), ("v", 2048, 512),
                ("q", 2048, 512),
                ("k", 2560, 512), ("v", 2560, 512),
                ("k", 3072, 512), ("v", 3072, 512),
                ("q", 2560, 512),
                ("k", 3584, 512), ("v", 3584, 512),
                ("q", 3072, 512), ("q", 3584, 512),
            ):
                fq.extend(range_closures(nm, ts, w))

            steps = [(b, qb, kb)
                     for b in range(B) for qb in range(NQB) for kb in range(NKB)]

            def qb_done_closure(pb, pqb):
                def qb_done():
                    emit_normalize(pb, pqb)
                    gqb = pb * NQB + pqb
                    emit_a2a_stage(gqb)
                    if gqb in (1, 3, 4, 5, 6):
                        emit_a2a(GQB_CH[gqb])
                return qb_done

            def push_oproj(c):
                if c not in og_sb:
                    fq.append(lambda: emit_og_load(c))
                for ob in range(DC):
                    fq.append(lambda ob=ob: emit_oproj_blk(c, ob))

            pending = None   # (b, qb, kb, p-tile) awaiting PV emission
            for idx, (b, qb, kb) in enumerate(steps):
                # 6/step while the first-qb production deadlines demand it,
                # then 1/step: spreading fillers evenly keeps the DVE queue
                # shallow, so qb-end normalize chains (the A2A critical path)
                # run within ~2us of the last PV instead of 15-40us late.
                budget = 6 if idx < 12 else 1
                p = emit_scores_exp(b, qb, kb)
                pump(budget)
                if pending is not None:
                    emit_pv(*pending)
                    pb, pqb, pkb = pending[0], pending[1], pending[2]
                    if pkb == NKB - 1:
                        # normalize + A2A staging/trigger ride the queue front
                        # so they run promptly after the qb finishes (and
                        # before the next qb's oe reuses the "pv" ring).
                        fq.appendleft(qb_done_closure(pb, pqb))
                pending = (b, qb, kb, p)

                # prefetch the first two og tiles during the last steps:
                # their A2As landed ~80us ago, the loads are pure sync-queue
                # DMAs (never in the PE FIFO), and having them resident lets
                # the tail o_proj start the instant the last qb finishes.
                if idx == 120:
                    emit_og_load(0)
                    emit_og_load(1)

            emit_pv(*pending)
            emit_normalize(1, 3, fast=True)
            emit_a2a_stage(7)
            emit_a2a(5)
            # ALL o_proj runs in the tail: the steps stay exp-paced (their
            # PE slack is too small for o_proj fillers anyway), and the
            # 35-40us of o_proj work exactly fills the flight time of the
            # last four collectives, so the PE never idles waiting on them.
            # og(c) is always loaded well after its A2A lands (cc0 ~136us,
            # first consumer ~224us; cc5 ~257us, consumer ~263us).
            push_oproj(0)
            push_oproj(1)
            while fq:
                fq.popleft()()

            # Tail chunks (2, 3 fill the PE while the final half-size A2A is
            # in flight; 4 follows it). Each accumulates all 8 output blocks
            # into one fin tile written by a single DMA.
            def emit_oproj_chunk_merged(c):
                w = CHUNKS[c][1]
                emit_og_load(c)
                finc = finp.tile([128, DC, w], F32, tag="finm", bufs=2,
                                 name=f"finm{c}")
                for ob in range(DC):
                    acc = psp.tile([128, w], F32, tag="pv", bufs=2,
                                   name=f"accm{c}{ob}")
                    for d in range(DC):
                        nc.tensor.matmul(
                            acc[:], wo_sb[:, d, ob * 128:(ob + 1) * 128],
                            og_sb[c][:, d, :],
                            start=(d == 0), stop=(d == DC - 1))
                    nc.vector.tensor_copy(finc[:, ob, :], acc[:])
                nc.sync.dma_start(
                    out[:, CB[c]:CB[c] + w].rearrange("(c p) m -> p c m", p=128),
                    finc[:])

            for c in (2, 3, 4, 5):
                emit_oproj_chunk_merged(c)

    nc.compile()
    return nc


def _get_nc():
    if "nc" not in _CACHED:
        _CACHED["nc"] = build()
    return _CACHED["nc"]


def _prep_w(Wm):
    # [D, M] (rows = contraction dim) -> SBUF layout [128, c, m] flattened
    # to [128, c*m], contiguous per partition.
    Dd, M = Wm.shape
    return np.ascontiguousarray(
        Wm.reshape(Dd // 128, 128, M).transpose(1, 0, 2).reshape(128, -1)
    ).astype(ml_dtypes.bfloat16)


def make_in_maps(x, Wq, Wk, Wv, Wo):
    xT = np.ascontiguousarray(
        np.asarray(x, dtype=np.float32).reshape(T, D).T).astype(ml_dtypes.bfloat16)
    woT = _prep_w(np.ascontiguousarray(np.asarray(Wo, dtype=np.float32).T))
    in_maps = []
    for c in range(N_CORES):
        r0, r1 = c * PC, (c + 1) * PC
        in_maps.append({
            "xT": xT,
            "wq": _prep_w(np.ascontiguousarray(np.asarray(Wq, np.float32)[r0:r1, :].T)),
            "wk": _prep_w(np.ascontiguousarray(np.asarray(Wk, np.float32)[r0:r1, :].T)),
            "wv": _prep_w(np.ascontiguousarray(np.asarray(Wv, np.float32)[r0:r1, :].T)),
            "wo": woT,
        })
    return in_maps


def assemble(outs):
    # outs[j]: [1024, 512] f32; chunk c's column block (base CB, width w)
    # holds tokens base_c + [w*j, w*(j+1)).
    chunks = [(0, 0, 128), (1024, 128, 128), (2048, 256, 64),
              (2560, 320, 64), (3072, 384, 64), (3584, 448, 64)]
    full = np.empty((T, D), dtype=np.float32)
    for j in range(N_CORES):
        o = outs[j]
        for base, cb, w in chunks:
            full[base + w * j:base + w * (j + 1), :] = o[:, cb:cb + w].T
    return np.ascontiguousarray(full.reshape(B, S, D))


def _spot_check(x, Wq, Wk, Wv, Wo, out,
                toks=(7, 1033, 2081, 2567, 3583, 4089)):  # one per A2A chunk
    """Recompute a few output rows in numpy straight from the inputs and
    compare. Catches the (rare) flaky-run/flaky-compile corruption so the
    caller can rebuild + retry instead of returning garbage."""
    xf = np.asarray(x, np.float32).reshape(T, D)
    rot = np.concatenate([np.arange(32, 64), np.arange(0, 32)])
    sgn = np.concatenate([-np.ones(32), np.ones(32)]).astype(np.float32)
    inv_freq = 1.0 / (10000.0 ** (np.arange(0, DH, 2, dtype=np.float64) / DH))
    fr = np.concatenate([np.arange(S)[:, None] * inv_freq[None, :]] * 2, axis=1)
    cos, sin = np.cos(fr).astype(np.float32), np.sin(fr).astype(np.float32)
    for b in range(B):
        bt = [t for t in toks if t // S == b]
        if not bt:
            continue
        xb = xf[b * S:(b + 1) * S]
        k = (xb @ Wk.T).reshape(S, H, DH)
        v = (xb @ Wv.T).reshape(S, H, DH)
        k = k * cos[:, None, :] + k[:, :, rot] * (sin * sgn)[:, None, :]
        for t in bt:
            s = t - b * S
            q = (xf[t] @ Wq.T).reshape(H, DH)
            q = q * cos[s][None, :] + q[:, rot] * (sin[s] * sgn)[None, :]
            sc = np.einsum("hd,shd->hs", q, k) / np.sqrt(np.float32(DH))
            p = np.exp(sc - sc.max(axis=1, keepdims=True))
            p /= p.sum(axis=1, keepdims=True)
            att = np.einsum("hs,shd->hd", p, v).reshape(D)
            exp_row = att @ Wo.T
            got = out.reshape(T, D)[t]
            if np.linalg.norm(got - exp_row) > 0.05 * np.linalg.norm(exp_row):
                return False
    return True


def kernel(x, Wq, Wk, Wv, Wo):
    in_maps = make_in_maps(x, Wq, Wk, Wv, Wo)
    for attempt in range(2):
        nc = _get_nc()
        res = run_bass_kernel_spmd(nc, in_maps, core_ids=list(range(N_CORES)))
        outs = [res.results[c]["out"] for c in range(N_CORES)]   # [1024, 512]
        full = assemble(outs).astype(np.float32)
        if _spot_check(x, Wq, Wk, Wv, Wo, full):
            return full
        _CACHED.clear()   # flaky run or compile: rebuild and retry once
    return full


if __name__ == "__main__":
    rng = np.random.default_rng(0)
    ins = {
        "x": rng.standard_normal((B, S, D), dtype=np.float32),
        "Wq": rng.standard_normal((D, D), dtype=np.float32) / 32,
        "Wk": rng.standard_normal((D, D), dtype=np.float32) / 32,
        "Wv": rng.standard_normal((D, D), dtype=np.float32) / 32,
        "Wo": rng.standard_normal((D, D), dtype=np.float32) / 32,
    }
    o = kernel(**ins)
    print("kernel out:", o.shape, o.dtype, float(np.abs(o).mean()))
